# revision 1
# baseline (speedup 1.0000x reference)
"""Qudit-CNOT permutation kernel for Trainium2 (8 NeuronCores).

Computes out[perm[k], :] = x[k, :] for a batch of state vectors.

Strategy (per the sharding hint): shard x column-wise across the 8 cores
(16 batch cols -> 2 per core); perm is identical for every core, so the
kernel is pure SPMD with no communication.

The CNOT permutation is block-structured: decomposed host-side into
maximal contiguous runs (src range -> dst range with stride 1), it is a
handful of large contiguous block moves (5 runs for the d=3, n=14,
ctrl=0, tgt=1 instance).  Each core's device program is then just a few
multi-MB DRAM->DRAM DMA copies — pure memory movement at HBM line rate,
which is the roofline for this memory-regime problem.
"""

import numpy as np

N_CORES = 8


def _build_copy_kernel(runs, n_elems):
    """Bass program: flat f32 in/out of n_elems; one DRAM->DRAM DMA per run.

    runs: list of (src_elem, dst_elem, len_elems) in flat element offsets.
    """
    import concourse.bass as bass
    import concourse.mybir as mybir

    nc = bass.Bass()
    xin = nc.declare_dram_parameter("x", [n_elems], mybir.dt.float32, isOutput=False)
    yout = nc.declare_dram_parameter("y", [n_elems], mybir.dt.float32, isOutput=True)

    with nc.Block() as block, nc.semaphore("dma_sem") as sem:

        @block.sync
        def _(sync):
            for src, dst, ln in runs:
                sync.dma_start(
                    out=yout[dst : dst + ln], in_=xin[src : src + ln]
                ).then_inc(sem, 16)
            sync.wait_ge(sem, 16 * len(runs))

    return nc


def kernel(x: np.ndarray, perm: np.ndarray) -> np.ndarray:
    from concourse.bass_utils import run_bass_kernel_spmd

    x = np.asarray(x)
    assert x.dtype == np.float32
    n_rows, batch = x.shape
    assert batch % N_CORES == 0
    cols = batch // N_CORES

    # Host-side: decompose the permutation into maximal contiguous runs.
    p = np.asarray(perm, dtype=np.int64).ravel()
    assert p.size == n_rows
    breaks = np.nonzero(np.diff(p) != 1)[0] + 1
    starts = np.concatenate(([0], breaks))
    ends = np.concatenate((breaks, [p.size]))
    if len(starts) > 256:
        raise NotImplementedError(
            f"perm has {len(starts)} contiguous runs; this kernel handles "
            "block-structured permutations only"
        )
    # Flat element offsets within each core's (n_rows, cols) shard.
    runs = [
        (int(s) * cols, int(p[s]) * cols, int(e - s) * cols)
        for s, e in zip(starts, ends)
    ]

    n_elems = n_rows * cols
    nc = _build_copy_kernel(runs, n_elems)

    in_maps = [
        {"x": np.ascontiguousarray(x[:, c * cols : (c + 1) * cols]).reshape(-1)}
        for c in range(N_CORES)
    ]
    res = run_bass_kernel_spmd(nc, in_maps, list(range(N_CORES))).results

    out = np.empty_like(x)
    for c in range(N_CORES):
        out[:, c * cols : (c + 1) * cols] = res[c]["y"].reshape(n_rows, cols)
    return out


# revision 2
# speedup vs baseline: 1.2026x; 1.2026x over previous
"""Qudit-CNOT permutation kernel for Trainium2 (8 NeuronCores).

Computes out[perm[k], :] = x[k, :] for a batch of state vectors
(x: (3^14, 16) f32; perm: the CNOT qudit-gate permutation).

Strategy (per the sharding hint): shard x column-wise across the 8 cores
(16 batch cols -> 2 per core); perm is identical for every core, so the
kernel is pure SPMD with no communication.

The CNOT permutation is block-structured: decomposed host-side into
maximal contiguous runs (src range -> dst range, stride 1), it is 5
large contiguous block moves for the d=3, n=14, ctrl=0, tgt=1 instance.
Each core's device program is pure DRAM->DRAM DMA — the memory roofline
for this problem.

Tuning (measured via NTFF profiles on trn2):
- One giant DMA per run drains at ~270 GB/s/direction; splitting into
  ~3 MB chunks spread over both HWDGE rings (SP 'sync' + ACT 'scalar')
  sustains ~318 GB/s/direction (~89% of the 358 GB/s per-NC HBM cap),
  ~127 us/core vs ~150-210 us for the unchunked version.
- Chunk sizes of 2.5/3.5/5/8 MB trip a deterministic walrus codegen
  failure; 3 MB (786432 f32 elems) compiles reliably — keep it fixed.
"""

import numpy as np

N_CORES = 8
CHUNK_ELEMS = 786432  # 3 MiB of f32 per DMA chunk


def _split_chunks(runs, chunk_elems=CHUNK_ELEMS):
    out = []
    for src, dst, ln in runs:
        off = 0
        while off < ln:
            c = min(chunk_elems, ln - off)
            out.append((src + off, dst + off, c))
            off += c
    return out


def _build_copy_kernel(runs, n_elems):
    """Bass program: flat f32 in/out of n_elems; chunked DRAM->DRAM DMA
    copies alternated across the two HWDGE rings (sync + scalar)."""
    import concourse.bass as bass
    import concourse.mybir as mybir

    chunks = _split_chunks(runs)
    a = chunks[0::2]
    b = chunks[1::2]

    nc = bass.Bass()
    xin = nc.declare_dram_parameter("x", [n_elems], mybir.dt.float32, isOutput=False)
    yout = nc.declare_dram_parameter("y", [n_elems], mybir.dt.float32, isOutput=True)

    def emit(eng, todo, sem):
        for src, dst, ln in todo:
            eng.dma_start(out=yout[dst : dst + ln], in_=xin[src : src + ln]).then_inc(
                sem, 16
            )

    with nc.Block() as block, nc.semaphore("dma_sem") as sem:

        @block.sync
        def _(sync):
            emit(sync, a, sem)
            sync.wait_ge(sem, 16 * len(chunks))

        @block.scalar
        def _(scalar):
            emit(scalar, b, sem)

    return nc


def kernel(x: np.ndarray, perm: np.ndarray) -> np.ndarray:
    from concourse.bass_utils import run_bass_kernel_spmd

    x = np.asarray(x)
    assert x.dtype == np.float32
    n_rows, batch = x.shape
    assert batch % N_CORES == 0
    cols = batch // N_CORES

    # Host-side: decompose the permutation into maximal contiguous runs.
    p = np.asarray(perm, dtype=np.int64).ravel()
    assert p.size == n_rows
    breaks = np.nonzero(np.diff(p) != 1)[0] + 1
    starts = np.concatenate(([0], breaks))
    ends = np.concatenate((breaks, [p.size]))
    if len(starts) > 256:
        raise NotImplementedError(
            f"perm has {len(starts)} contiguous runs; this kernel handles "
            "block-structured permutations only"
        )
    # Flat element offsets within each core's (n_rows, cols) shard.
    runs = [
        (int(s) * cols, int(p[s]) * cols, int(e - s) * cols)
        for s, e in zip(starts, ends)
    ]

    n_elems = n_rows * cols
    nc = _build_copy_kernel(runs, n_elems)

    in_maps = [
        {"x": np.ascontiguousarray(x[:, c * cols : (c + 1) * cols]).reshape(-1)}
        for c in range(N_CORES)
    ]
    res = run_bass_kernel_spmd(nc, in_maps, list(range(N_CORES))).results

    out = np.empty_like(x)
    for c in range(N_CORES):
        out[:, c * cols : (c + 1) * cols] = res[c]["y"].reshape(n_rows, cols)
    return out
